# revision 1
# baseline (speedup 1.0000x reference)
"""Trainium2 Bass kernel for nn_CriticNetwork (gnn_message_passing).

Math: the reference GNN does mean-aggregation over a complete graph with
self-loops, so every node of an env sees the identical per-env mean.  The
whole network collapses to per-env scalars:

  m_b  = mean over the 16 nodes of obs[b]                      [128]
  p_b  = relu(m_b @ W1 + b1) @ W2 + b2                         [64]
  a_b  = p_b . (Wfc @ (Wattn[:64] + Wattn[64:]))               scalar
  w_b  = sigmoid(leaky_relu(a_b, 0.01))                        scalar
  c_b  = p_b . Wv[:64] + bv                                    scalar
  P_bk = pi[b,k] . Wvy ;  A_bk = act[b,k] . Wvy                (Wvy = Wv[64:72])
  xv[b,j] = c_b + (PS_b + w_b*(AS_b-PS_b) - w_b*(A_bj-P_bj))/16
  out x[b*16+d, j] = xv[b,j]   (independent of d)
  out w[b*16+d, j] = w_b

Sharding: data-parallel over envs, 512 envs per core x 8 cores.

Per-core layout: local env e = 128*g + p (g = group, p = partition), so a
group's per-env scalars live one-per-partition and phase-B tiles hold env
column-blocks g with no cross-partition shuffles.
"""

import numpy as np
from contextlib import ExitStack

import concourse.bass as bass
import concourse.bacc as bacc
import concourse.tile as tile
from concourse import mybir
from concourse.bass_utils import run_bass_kernel_spmd

B, N, A = 4096, 16, 8
D_IN, H1, DP, DZ = 128, 64, 64, 64
NCORES = 8
BC = B // NCORES          # 512 envs per core
RC = BC * N               # 8192 obs rows per core
G = 4                     # env groups per core
GE = BC // G              # 128 envs per group
CW = 272                  # const tile width

F32 = mybir.dt.float32
ALU = mybir.AluOpType
AFT = mybir.ActivationFunctionType


def _build():
    nc = bacc.Bacc("TRN2", target_bir_lowering=False, debug=False)

    obs = nc.dram_tensor("obs", [RC, D_IN], F32, kind="ExternalInput")
    pol = nc.dram_tensor("pol", [RC, A], F32, kind="ExternalInput")
    act = nc.dram_tensor("act", [RC, A], F32, kind="ExternalInput")
    cst = nc.dram_tensor("cst", [128, CW], F32, kind="ExternalInput")
    xo = nc.dram_tensor("xo", [RC, N], F32, kind="ExternalOutput")
    wo = nc.dram_tensor("wo", [RC, N], F32, kind="ExternalOutput")

    with ExitStack() as ctx:
        tc = ctx.enter_context(tile.TileContext(nc))
        consts = ctx.enter_context(tc.tile_pool(name="consts", bufs=1))
        obsp = ctx.enter_context(tc.tile_pool(name="obsp", bufs=4))
        pap = ctx.enter_context(tc.tile_pool(name="pap", bufs=1))
        sb = ctx.enter_context(tc.tile_pool(name="sb", bufs=2))
        sbB = ctx.enter_context(tc.tile_pool(name="sbB", bufs=1))
        pmtp = ctx.enter_context(tc.tile_pool(name="pmtp", bufs=2, space="PSUM"))
        php = ctx.enter_context(tc.tile_pool(name="php", bufs=2, space="PSUM"))
        ppp = ctx.enter_context(tc.tile_pool(name="ppp", bufs=2, space="PSUM"))
        pacp = ctx.enter_context(tc.tile_pool(name="pacp", bufs=1, space="PSUM"))
        pwtp = ctx.enter_context(tc.tile_pool(name="pwtp", bufs=1, space="PSUM"))

        # cst on the scalar ring so the sync ring starts obs immediately
        cst_sb = consts.tile([128, CW], F32)
        nc.scalar.dma_start(out=cst_sb, in_=cst.ap())
        wvy8_sb = cst_sb[:, 0:8]            # Wvy on all partitions
        w1q_sb = cst_sb[:, 8:72]            # W1 / 16
        wq_sb = cst_sb[0:64, 72:74]         # W2 @ [wa | Wv[:64]]
        b1_sb = cst_sb[0:64, 138:139]
        biasq_sb = cst_sb[0:2, 140:141]     # [b2.wa, b2.Wv64 + bv]
        id2_sb = cst_sb[0:2, 142:144]       # eye(2)
        id128_sb = cst_sb[:, 144:272]       # eye(128)

        # obs rows for env e=128g+p: 16e..16e+15 -> group g tile [128, 2048]
        obs_v = obs.ap().rearrange("(g p nf) f -> g p (nf f)", p=128, nf=16)

        wc8 = sbB.tile([128, 8], F32)            # cols 0-3: w_g, 4-7: c_g

        # preload the sigmoid ACT table while DMAs stream
        warm = consts.tile([1, 1], F32)
        nc.scalar.activation(out=warm, in_=cst_sb[0:1, 0:1], func=AFT.Sigmoid)

        obs_tiles = []
        for g in range(G):
            obs_t = obsp.tile([128, 16 * 128], F32, name="obs_t")
            # all obs on ONE ring: per-ring transfers run serially at near
            # full HBM bandwidth, so group g's data lands ~3us apart and the
            # mean trees/chains pipeline behind the loads. (Splitting across
            # both rings makes every transfer finish late together.)
            nc.sync.dma_start(out=obs_t, in_=obs_v[g])
            obs_tiles.append(obs_t)

        # pol/act with interleaved env layout: partition p, block g = env 128g+p
        pa_view = lambda t: t.ap().rearrange("(g p n) a -> p g (n a)", p=128, n=16)
        pol_sb = pap.tile([128, G, N * A], F32)
        nc.sync.dma_start(out=pol_sb, in_=pa_view(pol))
        act_sb = pap.tile([128, G, N * A], F32)
        nc.sync.dma_start(out=act_sb, in_=pa_view(act))

        last_tree_inst = None
        for g in range(G):
            obs_t = obs_tiles[g]
            # sum over the 16 nodes: pairwise tree, all on DVE (POOL shares
            # SBUF ports with DVE - running both concurrently slows both)
            s1 = sb.tile([128, 1024], F32, name="s1")
            nc.vector.tensor_add(s1, obs_t[:, 0:1024], obs_t[:, 1024:2048])
            s2 = sb.tile([128, 512], F32, name="s2")
            nc.vector.tensor_add(s2, s1[:, 0:512], s1[:, 512:1024])
            s3 = sb.tile([128, 256], F32, name="s3")
            nc.vector.tensor_add(s3, s2[:, 0:256], s2[:, 256:512])
            meanS = sb.tile([128, 128], F32, name="meanS")
            last_tree_inst = nc.vector.tensor_add(meanS, s3[:, 0:128],
                                                  s3[:, 128:256])
            pmt = pmtp.tile([128, 128], F32, name="pmt")
            nc.tensor.transpose(pmt, meanS[:], id128_sb)
            meanT = sb.tile([128, GE], F32, name="meanT")
            nc.scalar.activation(out=meanT, in_=pmt, func=AFT.Copy)

            # chain: (sum/16) @ W1 + b1 -> relu -> @(W2@Wac) + biasq -> [a|c]
            ph = php.tile([64, GE], F32, name="ph")
            nc.tensor.matmul(ph, lhsT=w1q_sb, rhs=meanT[:],
                             start=True, stop=True)
            h_sb = sb.tile([64, GE], F32, name="h_sb")
            nc.scalar.activation(out=h_sb, in_=ph, func=AFT.Relu, bias=b1_sb)
            pac = pacp.tile([2, GE], F32, name="pac")
            nc.tensor.matmul(pac, lhsT=wq_sb, rhs=h_sb, start=True, stop=True)
            wc = sb.tile([2, GE], F32, name="wc")
            nc.scalar.activation(out=wc, in_=pac, func=AFT.Identity,
                                 bias=biasq_sb)
            lr = sb.tile([1, GE], F32, name="lr")
            nc.vector.scalar_tensor_tensor(out=lr, in0=wc[0:1, :], scalar=0.01,
                                           in1=wc[0:1, :], op0=ALU.mult,
                                           op1=ALU.max)
            nc.scalar.activation(out=wc[0:1, :], in_=lr, func=AFT.Sigmoid)
            # per-env scalars onto partitions: [2, 128] -> [128, 2]
            pwt = pwtp.tile([128, 2], F32, name="pwt")
            nc.tensor.transpose(pwt, wc[:], id2_sb)
            wc8_dst = bass.AP(tensor=wc8.tensor, offset=wc8.offset + g,
                              ap=[wc8.ap[0], [4, 2]])
            nc.vector.tensor_copy(wc8_dst, pwt)

        # ---- batched per-node dots: P = pi.Wvy, A = act.Wvy ----
        # Keep these off DVE's critical window: order them after the last
        # mean-tree op so group 3's tree isn't interleaved with them.
        wvyb = wvy8_sb.unsqueeze(1).unsqueeze(1).broadcast_to([128, G, 16, 8])
        tmP = sbB.tile([128, G, N * A], F32)
        i_tmP = nc.vector.tensor_tensor(
            out=tmP.rearrange("p g (r a) -> p g r a", a=8),
            in0=pol_sb.rearrange("p g (r a) -> p g r a", a=8),
            in1=wvyb, op=ALU.mult)
        tmA = sbB.tile([128, G, N * A], F32)
        i_tmA = nc.vector.tensor_tensor(
            out=tmA.rearrange("p g (r a) -> p g r a", a=8),
            in0=act_sb.rearrange("p g (r a) -> p g r a", a=8),
            in1=wvyb, op=ALU.mult)
        if last_tree_inst is not None:
            tile.add_dep_helper(i_tmP.ins, last_tree_inst.ins, sync=False,
                                reason="keep DVE free for the last mean tree")
            tile.add_dep_helper(i_tmA.ins, last_tree_inst.ins, sync=False,
                                reason="keep DVE free for the last mean tree")
        P64 = sbB.tile([128, 64], F32)
        nc.vector.reduce_sum(out=P64,
                             in_=tmP.rearrange("p g (r a) -> p (g r) a", a=8),
                             axis=mybir.AxisListType.X)
        A64 = sbB.tile([128, 64], F32)
        nc.vector.reduce_sum(out=A64,
                             in_=tmA.rearrange("p g (r a) -> p (g r) a", a=8),
                             axis=mybir.AxisListType.X)
        Q64 = sbB.tile([128, 64], F32)
        nc.vector.tensor_sub(Q64, A64, P64)
        PS4 = sbB.tile([128, 4], F32)
        nc.vector.reduce_sum(out=PS4, in_=P64.rearrange("p (i n) -> p i n", n=16),
                             axis=mybir.AxisListType.X)
        AS4 = sbB.tile([128, 4], F32)
        nc.vector.reduce_sum(out=AS4, in_=A64.rearrange("p (i n) -> p i n", n=16),
                             axis=mybir.AxisListType.X)
        QS4 = sbB.tile([128, 4], F32)
        nc.vector.tensor_sub(QS4, AS4, PS4)

        # ---- combine: xv = c + (PS + w*QS)/16 - (w/16)*Q ----
        wT4 = wc8[:, 0:4]
        cT4 = wc8[:, 4:8]
        negw4 = sbB.tile([128, 4], F32)
        nc.scalar.mul(negw4, wT4, -1.0 / N)
        t2 = sbB.tile([128, 4], F32)
        nc.vector.tensor_mul(t2, wT4, QS4)
        t3 = sbB.tile([128, 4], F32)
        nc.vector.tensor_add(t3, t2, PS4)
        base4 = sbB.tile([128, 4], F32)
        nc.vector.scalar_tensor_tensor(out=base4, in0=t3, scalar=1.0 / N,
                                       in1=cT4, op0=ALU.mult, op1=ALU.add)
        nwq = sbB.tile([128, 64], F32)
        nc.vector.tensor_tensor(out=nwq.rearrange("p (i n) -> p i n", n=16),
                                in0=Q64.rearrange("p (i n) -> p i n", n=16),
                                in1=negw4.unsqueeze(2).broadcast_to([128, 4, 16]),
                                op=ALU.mult)
        xv64 = sbB.tile([128, 64], F32)
        nc.vector.tensor_tensor(out=xv64.rearrange("p (i n) -> p i n", n=16),
                                in0=nwq.rearrange("p (i n) -> p i n", n=16),
                                in1=base4.unsqueeze(2).broadcast_to([128, 4, 16]),
                                op=ALU.add)
        # ---- outputs: env e = 128g+p occupies rows 16e..16e+15 ----
        # materialize full [128, 4*16*16] payloads, then 2 plain fast DMAs
        wbig = sbB.tile([128, G, 16, 16], F32)
        nc.vector.tensor_copy(
            wbig.rearrange("p g a b -> p g (a b)"),
            wT4.unsqueeze(2).broadcast_to([128, 4, 256]))
        xbig = sbB.tile([128, G, 16, 16], F32)
        nc.vector.tensor_copy(
            xbig, xv64.rearrange("p (g j) -> p g j", g=4).unsqueeze(2)
                .broadcast_to([128, 4, 16, 16]))
        xo_v = xo.ap().rearrange("(g p d) j -> p g (d j)", p=128, d=16)
        wo_v = wo.ap().rearrange("(g p d) j -> p g (d j)", p=128, d=16)
        # split across partition halves and both HWDGE rings so the final
        # transfers overlap
        nc.sync.dma_start(out=wo_v[0:64], in_=wbig[0:64])
        nc.scalar.dma_start(out=wo_v[64:128], in_=wbig[64:128])
        nc.sync.dma_start(out=xo_v[0:64], in_=xbig[0:64])
        nc.scalar.dma_start(out=xo_v[64:128], in_=xbig[64:128])

    nc.compile()
    return nc


_NC_CACHE = {}


def _get_nc():
    if "nc" not in _NC_CACHE:
        _NC_CACHE["nc"] = _build()
    return _NC_CACHE["nc"]


def _make_in_maps(inputs):
    obs = np.ascontiguousarray(np.asarray(inputs["obs"], np.float32))
    pol = np.ascontiguousarray(np.asarray(inputs["policies"], np.float32))
    act = np.ascontiguousarray(np.asarray(inputs["actions"], np.float32))
    W1 = np.asarray(inputs["W1"], np.float32)
    b1 = np.asarray(inputs["b1"], np.float32)
    W2 = np.asarray(inputs["W2"], np.float32)
    b2 = np.asarray(inputs["b2"], np.float32)
    Wfc = np.asarray(inputs["Wfc"], np.float32)
    Wattn = np.asarray(inputs["Wattn"], np.float32)
    Wv = np.asarray(inputs["Wv"], np.float32)
    bv = np.asarray(inputs["bv"], np.float32)

    wa = (Wfc @ (Wattn[:DZ] + Wattn[DZ:]))[:, 0]     # [64]
    wvy = Wv[DP:, 0]                                  # [8]

    wv64 = Wv[:DP, 0]
    cst = np.zeros((128, CW), np.float32)
    cst[:, 0:8] = wvy[None, :]
    cst[:, 8:72] = W1 / 16.0
    cst[0:64, 72] = W2 @ wa                  # Wq col 0
    cst[0:64, 73] = W2 @ wv64                # Wq col 1
    cst[0:64, 138] = b1
    cst[0, 140] = float(b2 @ wa)             # biasq
    cst[1, 140] = float(b2 @ wv64 + bv[0])
    cst[0:2, 142:144] = np.eye(2, dtype=np.float32)
    cst[:, 144:272] = np.eye(128, dtype=np.float32)

    in_maps = []
    for c in range(NCORES):
        in_maps.append({
            "obs": obs[c * RC:(c + 1) * RC],
            "pol": pol[c * RC:(c + 1) * RC],
            "act": act[c * RC:(c + 1) * RC],
            "cst": cst,
        })
    return in_maps


# Test-harness knobs (the grader just calls kernel() with defaults).
TRACE = False
TRACE_KWARGS = {}
LAST_RESULT = None


def kernel(**inputs):
    global LAST_RESULT
    nc = _get_nc()
    in_maps = _make_in_maps(inputs)
    res = run_bass_kernel_spmd(nc, in_maps, core_ids=list(range(NCORES)),
                               trace=TRACE, **TRACE_KWARGS)
    LAST_RESULT = res
    x = np.concatenate([r["xo"] for r in res.results], axis=0).reshape(B * N, N, 1)
    w = np.concatenate([r["wo"] for r in res.results], axis=0).reshape(B * N, N, 1)
    return x, w



# revision 2
# speedup vs baseline: 1.0058x; 1.0058x over previous
"""Trainium2 Bass kernel for nn_CriticNetwork (gnn_message_passing).

Math: the reference GNN does mean-aggregation over a complete graph with
self-loops, so every node of an env sees the identical per-env mean.  The
whole network collapses to per-env scalars:

  m_b  = mean over the 16 nodes of obs[b]                      [128]
  p_b  = relu(m_b @ W1 + b1) @ W2 + b2                         [64]
  a_b  = p_b . (Wfc @ (Wattn[:64] + Wattn[64:]))               scalar
  w_b  = sigmoid(leaky_relu(a_b, 0.01))                        scalar
  c_b  = p_b . Wv[:64] + bv                                    scalar
  P_bk = pi[b,k] . Wvy ;  A_bk = act[b,k] . Wvy                (Wvy = Wv[64:72])
  xv[b,j] = c_b + (PS_b + w_b*(QS_b))/16 - (w_b/16)*(A_bj-P_bj)
  out x[b*16+d, j] = xv[b,j]   (independent of d)
  out w[b*16+d, j] = w_b

Sharding: data-parallel over envs, 512 envs per core x 8 cores.

v2 engine split (vs v1 which ran everything on DVE):
  - obs mean-trees on DVE, bf16 below the first level (2x DVE modes)
  - per-env MLP chain on PE/ACT in bf16; the [2,32] "replication matmul"
    materializes [w*16 | c*16] per env on partitions in one PE op
  - P/A dot products + prefix scalars on GpSimd (otherwise idle)
  - outputs leave per group via broadcast-source DMAs (no [128,1024]
    materialized output tiles at all); obs streams on the sync ring while
    pol/act/consts ride the scalar ring
"""

import numpy as np
import ml_dtypes
from contextlib import ExitStack

import concourse.bass as bass
import concourse.bacc as bacc
import concourse.tile as tile
from concourse import mybir
from concourse.bass_utils import run_bass_kernel_spmd

B, N, A = 4096, 16, 8
D_IN, H1, DP, DZ = 128, 64, 64, 64
NCORES = 8
BC = B // NCORES          # 512 envs per core
RC = BC * N               # 8192 obs rows per core
G = 4                     # env groups per core (128 envs each)
GE = BC // G              # 128 envs per group

F32 = mybir.dt.float32
BF16 = mybir.dt.bfloat16
ALU = mybir.AluOpType
AFT = mybir.ActivationFunctionType


def _build():
    nc = bacc.Bacc("TRN2", target_bir_lowering=False, debug=False)

    obs = nc.dram_tensor("obs", [RC, D_IN], F32, kind="ExternalInput")
    pol = nc.dram_tensor("pol", [RC, A], F32, kind="ExternalInput")
    act = nc.dram_tensor("act", [RC, A], F32, kind="ExternalInput")
    cst = nc.dram_tensor("cst", [128, 48], F32, kind="ExternalInput")
    cstb = nc.dram_tensor("cstb", [128, 194], BF16, kind="ExternalInput")
    xo = nc.dram_tensor("xo", [RC, N], F32, kind="ExternalOutput")
    wo = nc.dram_tensor("wo", [RC, N], F32, kind="ExternalOutput")

    with ExitStack() as ctx:
        tc = ctx.enter_context(tile.TileContext(nc))
        consts = ctx.enter_context(tc.tile_pool(name="consts", bufs=1))
        obsp = ctx.enter_context(tc.tile_pool(name="obsp", bufs=1))
        trp = ctx.enter_context(tc.tile_pool(name="trp", bufs=2))
        pap = ctx.enter_context(tc.tile_pool(name="pap", bufs=1))
        gsp = ctx.enter_context(tc.tile_pool(name="gsp", bufs=1))
        smp = ctx.enter_context(tc.tile_pool(name="smp", bufs=1))
        pmtp = ctx.enter_context(tc.tile_pool(name="pmtp", bufs=2, space="PSUM"))
        php = ctx.enter_context(tc.tile_pool(name="php", bufs=2, space="PSUM"))
        pacp = ctx.enter_context(tc.tile_pool(name="pacp", bufs=2, space="PSUM"))
        pwp = ctx.enter_context(tc.tile_pool(name="pwp", bufs=2, space="PSUM"))

        # ---- input DMAs -------------------------------------------------
        # obs on the sync ring (q1): 8 half-group transfers, issued first.
        # half h of group g = nodes 8h..8h+7 of envs 128g+p (4KB/partition).
        obs_v = obs.ap().rearrange("(g p h nf) f -> g h p (nf f)",
                                   p=128, h=2, nf=8)
        obs_t = []
        for g in range(G):
            pair = []
            for h in range(2):
                t = obsp.tile([128, 8 * 128], F32, name=f"obs{g}{h}")
                nc.sync.dma_start(out=t, in_=obs_v[g][h])
                pair.append(t)
            obs_t.append(pair)

        # consts + pol/act on the scalar ring (q10)
        cst_sb = consts.tile([128, 48], F32)
        nc.scalar.dma_start(out=cst_sb, in_=cst.ap())
        cstb_sb = consts.tile([128, 194], BF16)
        nc.scalar.dma_start(out=cstb_sb, in_=cstb.ap())
        wvy_sb = cst_sb[:, 0:8]
        b1_sb = cst_sb[0:64, 8:9]
        biasq_sb = cst_sb[0:2, 9:10]
        rep2_sb = cst_sb[0:2, 10:42]        # [2,32]: row0 -> cols 0:16, row1 -> 16:32
        idb_sb = cstb_sb[:, 0:128]
        w1q_sb = cstb_sb[:, 128:192]
        wq_sb = cstb_sb[0:64, 192:194]

        pa_view = lambda t: t.ap().rearrange("(g p n) a -> p g (n a)", p=128, n=16)
        pol_sb = pap.tile([128, G, N * A], F32)
        nc.scalar.dma_start(out=pol_sb, in_=pa_view(pol))
        act_sb = pap.tile([128, G, N * A], F32)
        nc.scalar.dma_start(out=act_sb, in_=pa_view(act))

        # preload the sigmoid ACT table while DMAs stream
        warm = consts.tile([1, 1], F32)
        nc.scalar.activation(out=warm, in_=cst_sb[0:1, 0:1], func=AFT.Sigmoid)

        # per-env scalars, one column pair per group: cols g = w, 4+g = c
        wc8 = smp.tile([128, 8], F32)
        # output views: rows 16*(128g+p)+d
        xo_v = xo.ap().rearrange("(g p d) j -> g p d j", p=128, d=16)
        wo_v = wo.ap().rearrange("(g p d) j -> g p d j", p=128, d=16)

        w16 = [None] * G
        xv = [None] * G

        def tree_half(g, h):
            src = obs_t[g][h]
            h1 = trp.tile([128, 512], BF16, name="h1")
            nc.vector.tensor_add(h1, src[:, 0:512], src[:, 512:1024])
            h2 = trp.tile([128, 256], BF16, name="h2")
            nc.vector.tensor_add(h2, h1[:, 0:256], h1[:, 256:512])
            h3 = trp.tile([128, 128], BF16, name=f"h3_{h}")
            nc.vector.tensor_add(h3, h2[:, 0:128], h2[:, 128:256])
            return h3

        def tree_join(g, a3, b3):
            meanS = trp.tile([128, 128], BF16, name="meanS")
            nc.vector.tensor_add(meanS, a3, b3)
            return meanS

        def chain(g, meanS):
            # meanS[p=env, f] --T--> [f, env] -> W1 -> relu -> wq -> [a|c]
            pmt = pmtp.tile([128, 128], BF16, name="pmt")
            nc.tensor.transpose(pmt, meanS[:], idb_sb)
            meanT = trp.tile([128, GE], BF16, name="meanT")
            nc.scalar.activation(out=meanT, in_=pmt, func=AFT.Copy)
            ph = php.tile([64, GE], F32, name="ph")
            nc.tensor.matmul(ph, lhsT=w1q_sb, rhs=meanT[:], start=True, stop=True)
            h_sb = trp.tile([64, GE], BF16, name="h_sb")
            nc.scalar.activation(out=h_sb, in_=ph, func=AFT.Relu, bias=b1_sb)
            pac = pacp.tile([2, GE], F32, name="pac")
            nc.tensor.matmul(pac, lhsT=wq_sb, rhs=h_sb, start=True, stop=True)
            wc = trp.tile([2, GE], F32, name="wc")
            nc.scalar.activation(out=wc, in_=pac, func=AFT.Identity, bias=biasq_sb)
            return wc

        def post(g, wc):
            # w = sigmoid(leaky(a)); [2,128] -> per-env [w*16 | c*16] via rep2
            lr = trp.tile([1, GE], F32, name="lr")
            nc.vector.scalar_tensor_tensor(out=lr, in0=wc[0:1, :], scalar=0.01,
                                           in1=wc[0:1, :], op0=ALU.mult,
                                           op1=ALU.max)
            nc.scalar.activation(out=wc[0:1, :], in_=lr, func=AFT.Sigmoid)
            pw16 = pwp.tile([128, 32], F32, name="pw16")
            nc.tensor.matmul(pw16, lhsT=wc[:], rhs=rep2_sb, start=True, stop=True)
            w16[g] = smp.tile([128, 16], F32, name=f"w16_{g}")
            nc.scalar.activation(out=w16[g], in_=pw16[:, 0:16], func=AFT.Copy)
            wc8_dst = bass.AP(tensor=wc8.tensor, offset=wc8.offset + g,
                              ap=[wc8.ap[0], [4, 2]])
            pw2 = bass.AP(tensor=pw16.tensor, offset=pw16.offset,
                          ap=[pw16.ap[0], [16, 2]])
            nc.scalar.activation(out=wc8_dst, in_=pw2, func=AFT.Copy)
            nc.sync.dma_start(out=wo_v[g],
                              in_=w16[g].unsqueeze(1).broadcast_to([128, 16, 16]))

        def combine(g, eng):
            # xv = base + negw*Q ; base = (w*QS + PS)/16 + c ; negw = -w/16
            s1 = smp.tile([128, 1], F32, name=f"s1_{g}")
            eng.tensor_tensor(out=s1, in0=QS4[:, g:g + 1], in1=wc8[:, g:g + 1],
                              op=ALU.mult)
            eng.tensor_add(s1, s1, PS4[:, g:g + 1])
            base = smp.tile([128, 1], F32, name=f"base_{g}")
            eng.tensor_scalar_mul(base, s1, 1.0 / N)
            eng.tensor_add(base, base, wc8[:, 4 + g:5 + g])
            negw = smp.tile([128, 1], F32, name=f"negw_{g}")
            eng.tensor_scalar_mul(negw, wc8[:, g:g + 1], -1.0 / N)
            xv[g] = smp.tile([128, 16], F32, name=f"xv_{g}")
            nc.vector.scalar_tensor_tensor(out=xv[g], in0=Q64[:, 16 * g:16 * g + 16],
                                           scalar=negw[:, 0:1],
                                           in1=base.broadcast_to([128, 16]),
                                           op0=ALU.mult, op1=ALU.add)
            nc.sync.dma_start(out=xo_v[g],
                              in_=xv[g].unsqueeze(1).broadcast_to([128, 16, 16]))

        # ---- group 0/1 trees, chains -----------------------------------
        a3 = tree_half(0, 0); b3 = tree_half(0, 1); m0 = tree_join(0, a3, b3)
        wc0 = chain(0, m0)
        a3 = tree_half(1, 0); b3 = tree_half(1, 1); m1 = tree_join(1, a3, b3)
        post(0, wc0)
        wc1 = chain(1, m1)

        # ---- gpsimd P/A block (runs behind pol/act arrival) ------------
        wvyb = wvy_sb.unsqueeze(1).unsqueeze(1).broadcast_to([128, G, 16, 8])
        tmP = gsp.tile([128, G, N * A], F32)
        nc.gpsimd.tensor_tensor(out=tmP.rearrange("p g (r a) -> p g r a", a=8),
                                in0=pol_sb.rearrange("p g (r a) -> p g r a", a=8),
                                in1=wvyb, op=ALU.mult)
        tmA = gsp.tile([128, G, N * A], F32)
        nc.gpsimd.tensor_tensor(out=tmA.rearrange("p g (r a) -> p g r a", a=8),
                                in0=act_sb.rearrange("p g (r a) -> p g r a", a=8),
                                in1=wvyb, op=ALU.mult)

        def gtree8(tm, nm):
            v = tm.rearrange("p g (r a) -> p (g r) a", a=8)
            t1 = gsp.tile([128, 64, 4], F32, name=f"t1{nm}")
            nc.gpsimd.tensor_add(t1, v[:, :, 0:4], v[:, :, 4:8])
            t2 = gsp.tile([128, 64, 2], F32, name=f"t2{nm}")
            nc.gpsimd.tensor_add(t2, t1[:, :, 0:2], t1[:, :, 2:4])
            t3 = gsp.tile([128, 64], F32, name=f"t3{nm}")
            nc.gpsimd.tensor_add(t3, t2[:, :, 0:1].rearrange("p i o -> p (i o)"),
                                 t2[:, :, 1:2].rearrange("p i o -> p (i o)"))
            return t3

        P64 = gtree8(tmP, "P")
        A64 = gtree8(tmA, "A")
        Q64 = gsp.tile([128, 64], F32)
        nc.gpsimd.tensor_sub(Q64, A64, P64)

        def gtree16(t64, nm):
            v = t64.rearrange("p (i n) -> p i n", n=16)
            u1 = gsp.tile([128, 4, 8], F32, name=f"u1{nm}")
            nc.gpsimd.tensor_add(u1, v[:, :, 0:8], v[:, :, 8:16])
            u2 = gsp.tile([128, 4, 4], F32, name=f"u2{nm}")
            nc.gpsimd.tensor_add(u2, u1[:, :, 0:4], u1[:, :, 4:8])
            u3 = gsp.tile([128, 4, 2], F32, name=f"u3{nm}")
            nc.gpsimd.tensor_add(u3, u2[:, :, 0:2], u2[:, :, 2:4])
            u4 = gsp.tile([128, 4], F32, name=f"u4{nm}")
            nc.gpsimd.tensor_add(u4, u3[:, :, 0:1].rearrange("p i o -> p (i o)"),
                                 u3[:, :, 1:2].rearrange("p i o -> p (i o)"))
            return u4

        PS4 = gtree16(P64, "P")
        QS4 = gtree16(Q64, "Q")

        # ---- groups 2/3 interleaved with posts/combines ----------------
        a3 = tree_half(2, 0); b3 = tree_half(2, 1); m2 = tree_join(2, a3, b3)
        post(1, wc1)
        wc2 = chain(2, m2)
        a33 = tree_half(3, 0)
        post(2, wc2)
        combine(0, nc.gpsimd)
        combine(1, nc.gpsimd)
        combine(2, nc.gpsimd)
        b33 = tree_half(3, 1)
        m3 = tree_join(3, a33, b33)
        wc3 = chain(3, m3)
        post(3, wc3)
        combine(3, nc.vector)

    nc.compile()
    return nc


_NC_CACHE = {}


def _get_nc():
    if "nc" not in _NC_CACHE:
        _NC_CACHE["nc"] = _build()
    return _NC_CACHE["nc"]


def _make_in_maps(inputs):
    obs = np.ascontiguousarray(np.asarray(inputs["obs"], np.float32))
    pol = np.ascontiguousarray(np.asarray(inputs["policies"], np.float32))
    act = np.ascontiguousarray(np.asarray(inputs["actions"], np.float32))
    W1 = np.asarray(inputs["W1"], np.float32)
    b1 = np.asarray(inputs["b1"], np.float32)
    W2 = np.asarray(inputs["W2"], np.float32)
    b2 = np.asarray(inputs["b2"], np.float32)
    Wfc = np.asarray(inputs["Wfc"], np.float32)
    Wattn = np.asarray(inputs["Wattn"], np.float32)
    Wv = np.asarray(inputs["Wv"], np.float32)
    bv = np.asarray(inputs["bv"], np.float32)

    wa = (Wfc @ (Wattn[:DZ] + Wattn[DZ:]))[:, 0]     # [64]
    wvy = Wv[DP:, 0]                                  # [8]
    wv64 = Wv[:DP, 0]

    cst = np.zeros((128, 48), np.float32)
    cst[:, 0:8] = wvy[None, :]
    cst[0:64, 8] = b1
    cst[0, 9] = float(b2 @ wa)
    cst[1, 9] = float(b2 @ wv64 + bv[0])
    cst[0, 10:26] = 1.0                      # rep2 row0 -> w slots
    cst[1, 26:42] = 1.0                      # rep2 row1 -> c slots
    cstb = np.zeros((128, 194), np.float32)
    cstb[:, 0:128] = np.eye(128, dtype=np.float32)
    cstb[:, 128:192] = W1 / 16.0
    cstb[0:64, 192] = W2 @ wa
    cstb[0:64, 193] = W2 @ wv64
    cstb = cstb.astype(ml_dtypes.bfloat16)

    in_maps = []
    for c in range(NCORES):
        in_maps.append({
            "obs": obs[c * RC:(c + 1) * RC],
            "pol": pol[c * RC:(c + 1) * RC],
            "act": act[c * RC:(c + 1) * RC],
            "cst": cst,
            "cstb": cstb,
        })
    return in_maps


# Test-harness knobs (the grader just calls kernel() with defaults).
TRACE = False
TRACE_KWARGS = {}
LAST_RESULT = None


def kernel(**inputs):
    global LAST_RESULT
    nc = _get_nc()
    in_maps = _make_in_maps(inputs)
    res = run_bass_kernel_spmd(nc, in_maps, core_ids=list(range(NCORES)),
                               trace=TRACE, **TRACE_KWARGS)
    LAST_RESULT = res
    x = np.concatenate([r["xo"] for r in res.results], axis=0).reshape(B * N, N, 1)
    w = np.concatenate([r["wo"] for r in res.results], axis=0).reshape(B * N, N, 1)
    return x, w
